# revision 7
# baseline (speedup 1.0000x reference)
"""Trainium2 Bass kernel for causal multi-head attention (GPT-style block).

Reference computation (per batch b):
    qkv = x @ w_attn + b_attn ; q,k,v = split(qkv)
    per head: S = q k^T / sqrt(64); causal mask; P = softmax(S); a = P v
    out = concat_heads(a) @ w_proj + b_proj

Shapes: x (2, 2048, 1024), 16 heads, head_dim 64.

Sharding: 8 cores = 2 batches x 4 head-groups (4 heads each).  Tensor
parallel over heads: each core computes QKV projection for its 4 heads
(column slice of w_attn), full causal attention for those heads, and its
partial output projection (row slice of w_proj).  Host sums the 4
head-group partials per batch and adds b_proj.

On-chip layouts (per core, T=2048, CW=256=4*64):
    xT      [1024, T]   x transposed (host-prepped), streamed per q-chunk
    Q^T,K^T [CW, T]     projections with head-channel on partitions
    V_aug   [T, 4*65]   V natural layout + ones column per head (the ones
                        column makes the PV matmul also produce the softmax
                        denominator as output row 64)
    S^T     [k, q]      scores transposed: PSUM [128, 512] per (k-tile,
                        q-chunk); P^T = exp(S^T/8) directly feeds PV as the
                        moving operand - no transposes needed in the hot loop.
Causal masking: off-diagonal-band blocks are skipped entirely; band blocks
are zeroed post-exp with gpsimd.affine_select (iota predicate), which is
exact (exp(s)*0) and runs on the otherwise idle GPSIMD engine.
All matmuls run as float32r (full-rate on TRN2 for moving dim >= 256).
"""

import sys

sys.path.insert(0, "/opt/trn_rl_repo")

import numpy as np

import concourse.bacc as bacc
import concourse.mybir as mybir
import concourse.tile as tile
from concourse.bass_utils import run_bass_kernel_spmd

F32 = mybir.dt.float32
F32R = mybir.dt.float32r

B = 2
T = 2048
NX = 1024
H = 16
HD = 64
NCORES = 8
NHG = 4          # head groups (cores per batch)
NH = 4           # heads per core
CW = NH * HD     # 256 channel width per core
QC = 512         # q-chunk (moving dim)
NQC = T // QC    # 4
KT = 128         # k-tile
VW = HD + 1      # 65: V columns + ones column


def _build():
    nc = bacc.Bacc("TRN2", target_bir_lowering=False, debug=False,
                   num_devices=NCORES)
    xT_d = nc.dram_tensor("xT", [NX, T], F32R, kind="ExternalInput")
    wq_d = nc.dram_tensor("wq", [NX, CW], F32R, kind="ExternalInput")
    wk_d = nc.dram_tensor("wk", [NX, CW], F32R, kind="ExternalInput")
    wv_d = nc.dram_tensor("wv", [NX, CW], F32R, kind="ExternalInput")
    bias_d = nc.dram_tensor("bias", [128, 6], F32, kind="ExternalInput")
    wp_d = nc.dram_tensor("wp", [CW, NX], F32R, kind="ExternalInput")
    ident_d = nc.dram_tensor("ident", [128, 128], F32R, kind="ExternalInput")
    vones_d = nc.dram_tensor("vones", [128, 16 * NH], F32R, kind="ExternalInput")
    out_d = nc.dram_tensor("out_p", [T, NX], F32, kind="ExternalOutput")

    Ident = mybir.ActivationFunctionType.Identity
    Exp = mybir.ActivationFunctionType.Exp

    with tile.TileContext(nc) as tc:
        with (
            tc.tile_pool(name="pers", bufs=1) as pers,
            tc.tile_pool(name="xin", bufs=2) as xin,
            tc.tile_pool(name="ps", bufs=1, space="PSUM") as psum,
            tc.tile_pool(name="ptp", bufs=4) as ptp,
            tc.tile_pool(name="stg", bufs=4) as stg,
            tc.tile_pool(name="op", bufs=4) as op,
            tc.tile_pool(name="rp", bufs=2) as rp,
        ):
            def bank(i, shape, dtype=F32):
                return psum.tile(shape, dtype, tag=f"bank{i}", bufs=1, name=f"bank{i}")

            # ---- persistent tiles + constant loads ----
            wqkv = pers.tile([128, 8, 3 * CW], F32R, tag="wqkv")
            for w_d, off in ((wq_d, 0), (wk_d, CW), (wv_d, 2 * CW)):
                nc.sync.dma_start(
                    wqkv[:, :, off:off + CW],
                    w_d.ap().rearrange("(j p) c -> p j c", p=128),
                )
            wp = pers.tile([128, 2, NX], F32R, tag="wp")
            nc.sync.dma_start(wp[:], wp_d.ap().rearrange("(c p) n -> p c n", p=128))
            bias = pers.tile([128, 6], F32, tag="bias")
            nc.sync.dma_start(bias[:], bias_d.ap())
            ident = pers.tile([128, 128], F32R, tag="ident")
            nc.sync.dma_start(ident[:], ident_d.ap())

            QT = [pers.tile([128, T], F32R, tag=f"qt{i}", name=f"qt{i}") for i in range(2)]
            KTs = [pers.tile([128, T], F32R, tag=f"kt{i}", name=f"kt{i}") for i in range(2)]
            anorm = [pers.tile([128, T], F32R, tag=f"an{i}", name=f"an{i}") for i in range(2)]
            vaug = pers.tile([128, T // KT, NH * VW], F32R, tag="vaug")
            nc.sync.dma_start(
                vaug[:].rearrange("p t (h w) -> p t h w", h=NH)[:, :, :, HD:HD + 1],
                vones_d.ap().rearrange("p (t h w) -> p t h w", t=16, h=NH),
            )

            xT_r = xT_d.ap().rearrange("(j p) t -> j p t", p=128)

            def load_x(qq):
                xt = xin.tile([128, 8, QC], F32R, tag="xt")
                for j in range(8):
                    nc.sync.dma_start(xt[:, j, :],
                                      xT_r[j][:, qq * QC:(qq + 1) * QC])
                return xt

            def qkv_proj(qq, xt):
                """Project q-chunk qq -> QT/KT chunks + V^T psum, evac."""
                pq = [bank(0, [128, QC]), bank(1, [128, QC])]
                pk = [bank(2, [128, QC]), bank(3, [128, QC])]
                pv = [bank(4, [128, QC]), bank(5, [128, QC])]
                for j in range(8):
                    rhs = xt[:, j, :]
                    for c2 in range(2):
                        for grp, off in ((pq, 0), (pk, CW), (pv, 2 * CW)):
                            lhsT = wqkv[:, j, off + c2 * 128:off + (c2 + 1) * 128]
                            nc.tensor.matmul(grp[c2][:], lhsT, rhs,
                                             start=(j == 0), stop=(j == 7))
                cs = slice(qq * QC, (qq + 1) * QC)
                vstages = []
                for c2 in range(2):
                    nc.scalar.activation(QT[c2][:, cs], pq[c2][:], Ident,
                                         bias=bias[:, c2:c2 + 1])
                    nc.scalar.activation(KTs[c2][:, cs], pk[c2][:], Ident,
                                         bias=bias[:, 2 + c2:3 + c2])
                    vs = stg.tile([128, QC], F32R, tag="vstage")
                    nc.scalar.activation(vs[:], pv[c2][:], Ident,
                                         bias=bias[:, 4 + c2:5 + c2])
                    vstages.append(vs)
                return vstages

            def v_transpose(qq, vstages):
                """PE-transpose V^T chunk -> V natural into vaug tiles."""
                for c2 in range(2):
                    vs = vstages[c2]
                    for blk in range(4):
                        pt_ps = bank(6 + (blk % 2), [128, 128], F32R)
                        nc.tensor.transpose(pt_ps[:],
                                            vs[:, blk * 128:(blk + 1) * 128],
                                            ident[:])
                        tt = qq * 4 + blk
                        dst = vaug[:, tt, c2 * 2 * VW:c2 * 2 * VW + 2 * VW]
                        dst = dst.rearrange("p (h w) -> p h w", h=2)[:, :, 0:HD]
                        src = pt_ps[:].rearrange("p (h w) -> p h w", h=2)
                        nc.vector.tensor_copy(dst, src)

            def attention(h, qq):
                """One head, one q-chunk: S^T blocks -> exp -> PV accumulate."""
                c2, hh = divmod(h, 2)
                nk = 4 * qq + 4
                rows = slice(64 * hh, 64 * hh + 64)
                qs = slice(qq * QC, (qq + 1) * QC)
                pa = bank(4 + (h % 2), [VW, QC])
                pts = {}
                LA = 3

                def s_block(kk):
                    ps_s = bank(kk % 4, [128, QC])
                    lhsT = KTs[c2][rows, kk * KT:(kk + 1) * KT]
                    rhs = QT[c2][rows, qs]
                    nc.tensor.matmul(ps_s[:], lhsT,
                                     rhs, start=True, stop=True)
                    pt = ptp.tile([128, QC], F32R, tag="pt")
                    nc.scalar.activation(pt[:], ps_s[:], Exp, scale=0.125)
                    if kk >= 4 * qq:  # diagonal band: zero entries with k > q
                        nc.gpsimd.affine_select(
                            pt[:], pt[:], pattern=[[1, QC]],
                            compare_op=mybir.AluOpType.is_ge, fill=0.0,
                            base=qq * QC - kk * KT, channel_multiplier=-1)
                    pts[kk] = pt

                def pv_block(kk):
                    lhsT = vaug[:, kk, h * VW:(h + 1) * VW]
                    nc.tensor.matmul(pa[:], lhsT,
                                     pts.pop(kk)[:],
                                     start=(kk == 0), stop=(kk == nk - 1))

                for kk in range(min(LA, nk)):
                    s_block(kk)
                for kk in range(nk):
                    if kk + LA < nk:
                        s_block(kk + LA)
                    pv_block(kk)

                # normalize: row VW-1 of pa is the softmax denominator
                recip = rp.tile([1, QC], F32, tag="recip")
                nc.vector.reciprocal(recip[:], pa[HD:HD + 1, :])
                rbc = rp.tile([64, QC], F32, tag="rbc")
                nc.gpsimd.partition_broadcast(rbc[:], recip[:])
                nc.vector.tensor_mul(anorm[c2][rows, qs], pa[0:HD, :], rbc[:])

            def c_proj(qq):
                """Output projection for t-rows [qq*QC, (qq+1)*QC)."""
                for i in range(4):
                    tt = qq * 4 + i
                    for nxc in range(2):
                        po = bank(6 + (i * 2 + nxc) % 2, [128, QC])
                        for c2 in range(2):
                            nc.tensor.matmul(
                                po[:],
                                anorm[c2][:, tt * 128:(tt + 1) * 128],
                                wp[:, c2, nxc * QC:(nxc + 1) * QC],
                                start=(c2 == 0), stop=(c2 == 1))
                        ot = op.tile([128, QC], F32, tag="ot")
                        if (i * 2 + nxc) % 2 == 0:
                            nc.scalar.copy(ot[:], po[:])
                        else:
                            nc.vector.tensor_copy(ot[:], po[:])
                        nc.sync.dma_start(
                            out_d.ap()[tt * 128:(tt + 1) * 128,
                                       nxc * QC:(nxc + 1) * QC], ot[:])

            # ---- main pipeline over q-chunks ----
            xt = load_x(0)
            vs_prev = None
            for qq in range(NQC):
                vstages = qkv_proj(qq, xt)
                if qq + 1 < NQC:
                    xt = load_x(qq + 1)
                v_transpose(qq, vstages)
                for h in range(NH):
                    attention(h, qq)
                c_proj(qq)

    nc.compile()
    return nc


_CACHE = {}


def _get_nc():
    if "nc" not in _CACHE:
        _CACHE["nc"] = _build()
    return _CACHE["nc"]


def kernel(x, w_attn, b_attn, w_proj, b_proj):
    x = np.asarray(x, dtype=np.float32)
    w_attn = np.asarray(w_attn, dtype=np.float32)
    b_attn = np.asarray(b_attn, dtype=np.float32)
    w_proj = np.asarray(w_proj, dtype=np.float32)
    b_proj = np.asarray(b_proj, dtype=np.float32)

    ident = np.eye(128, dtype=np.float32)
    in_maps = []
    for core in range(NCORES):
        b, hg = divmod(core, NHG)
        cols = slice(hg * CW, (hg + 1) * CW)
        bias = np.empty((128, 6), dtype=np.float32)
        for qkv_i in range(3):
            bseg = b_attn[qkv_i * NX:][cols]
            bias[:, 2 * qkv_i] = bseg[:128]
            bias[:, 2 * qkv_i + 1] = bseg[128:]
        in_maps.append({
            "xT": np.ascontiguousarray(x[b].T),
            "wq": np.ascontiguousarray(w_attn[:, cols]),
            "wk": np.ascontiguousarray(w_attn[:, NX:][:, cols]),
            "wv": np.ascontiguousarray(w_attn[:, 2 * NX:][:, cols]),
            "bias": bias,
            "wp": np.ascontiguousarray(w_proj[cols, :]),
            "ident": ident,
            "vones": np.ones((128, 64), dtype=np.float32),
        })

    nc = _get_nc()
    res = run_bass_kernel_spmd(nc, in_maps, core_ids=list(range(NCORES)))
    _CACHE["last_res"] = res
    out = np.empty((B, T, NX), dtype=np.float32)
    for b in range(B):
        acc = res.results[b * NHG]["out_p"].astype(np.float32)
        for hg in range(1, NHG):
            acc = acc + res.results[b * NHG + hg]["out_p"]
        out[b] = acc + b_proj
    return out


# revision 9
# speedup vs baseline: 1.0808x; 1.0808x over previous
"""Trainium2 Bass kernel for causal multi-head attention (GPT-style block).

Reference computation (per batch b):
    qkv = x @ w_attn + b_attn ; q,k,v = split(qkv)
    per head: S = q k^T / sqrt(64); causal mask; P = softmax(S); a = P v
    out = concat_heads(a) @ w_proj + b_proj

Shapes: x (2, 2048, 1024), 16 heads, head_dim 64.

Sharding: 8 cores = 2 batches x 4 head-groups (4 heads each).  Tensor
parallel over heads: each core computes the QKV projection for its 4 heads
(column slice of w_attn), full causal attention for those heads, and its
partial output projection (row slice of w_proj).  Host sums the 4
head-group partials per batch and adds b_proj.

On-chip layouts (per core, T=2048, CW=256=4*64):
    xT      [1024, T]   x transposed (host-prepped bf16), streamed per chunk
    Q^T,K^T [CW, T]     projections with head-channel on partitions (bf16)
    V_aug   [T, 4*65]   V natural layout + ones column per head (the ones
                        column makes the PV matmul also produce the softmax
                        denominator as output row 64)
    S^T     [k, q]      scores transposed: PSUM [128, 512] per (k-tile,
                        q-chunk); P^T = exp(S^T/8) directly feeds PV as the
                        moving operand - no transposes in the hot loop.
Causal masking: off-band blocks are skipped entirely; band blocks are
zeroed post-exp with gpsimd.affine_select (exact: exp(s)*0) on the
otherwise idle GPSIMD engine.  All matmuls are bf16 (1 cycle/row on the
PE; fp32r measured ~1.85 cyc/row on this silicon).  Attention processes
head PAIRS interleaved: the two heads' S^T matmuls land on disjoint
PE row-groups (contract dim 64, base partitions 0/64) and run
concurrently, and the extra independent work keeps the PE dense across
the exp->mask->PV dependency chain (HAM stays warm).
"""

import sys

sys.path.insert(0, "/opt/trn_rl_repo")

import numpy as np
import ml_dtypes

import concourse.bacc as bacc
import concourse.mybir as mybir
import concourse.tile as tile
from concourse.bass_utils import run_bass_kernel_spmd

F32 = mybir.dt.float32
F32R = mybir.dt.float32r
BF16 = mybir.dt.bfloat16
NP_BF16 = np.dtype(ml_dtypes.bfloat16)

B = 2
T = 2048
NX = 1024
H = 16
HD = 64
NCORES = 8
NHG = 4          # head groups (cores per batch)
NH = 4           # heads per core
CW = NH * HD     # 256 channel width per core
QC = 512         # q-chunk (moving dim)
NQC = T // QC    # 4
KT = 128         # k-tile
VW = HD + 1      # 65: V columns + ones column


def _build():
    nc = bacc.Bacc("TRN2", target_bir_lowering=False, debug=False,
                   num_devices=NCORES)
    xT_d = nc.dram_tensor("xT", [NX, T], BF16, kind="ExternalInput")
    wq_d = nc.dram_tensor("wq", [NX, CW], BF16, kind="ExternalInput")
    wk_d = nc.dram_tensor("wk", [NX, CW], BF16, kind="ExternalInput")
    wv_d = nc.dram_tensor("wv", [NX, CW], BF16, kind="ExternalInput")
    bias_d = nc.dram_tensor("bias", [128, 6], F32, kind="ExternalInput")
    wp_d = nc.dram_tensor("wp", [CW, NX], BF16, kind="ExternalInput")
    ident_d = nc.dram_tensor("ident", [128, 128], F32R, kind="ExternalInput")
    vones_d = nc.dram_tensor("vones", [128, 16 * NH], BF16, kind="ExternalInput")
    out_d = nc.dram_tensor("out_p", [T, NX], F32, kind="ExternalOutput")

    Ident = mybir.ActivationFunctionType.Identity
    Exp = mybir.ActivationFunctionType.Exp

    with tile.TileContext(nc) as tc:
        with (
            tc.tile_pool(name="pers", bufs=1) as pers,
            tc.tile_pool(name="xin", bufs=2) as xin,
            tc.tile_pool(name="ps", bufs=1, space="PSUM") as psum,
            tc.tile_pool(name="ptp", bufs=8) as ptp,
            tc.tile_pool(name="stg", bufs=4) as stg,
            tc.tile_pool(name="op", bufs=4) as op,
            tc.tile_pool(name="rp", bufs=4) as rp,
        ):
            def bank(i, shape, dtype=F32):
                return psum.tile(shape, dtype, tag=f"bank{i}", bufs=1,
                                 name=f"bank{i}")

            # ---- persistent tiles; load order: first-needed first ----
            wqkv = pers.tile([128, 8, 3 * CW], BF16, tag="wqkv")
            for w_d, off in ((wq_d, 0), (wk_d, CW), (wv_d, 2 * CW)):
                nc.sync.dma_start(
                    wqkv[:, :, off:off + CW],
                    w_d.ap().rearrange("(j p) c -> p j c", p=128),
                )
            xT_r = xT_d.ap().rearrange("(j p) t -> j p t", p=128)

            def load_x(qq):
                xt = xin.tile([128, 8, QC], BF16, tag="xt")
                for j in range(8):
                    nc.sync.dma_start(xt[:, j, :],
                                      xT_r[j][:, qq * QC:(qq + 1) * QC])
                return xt

            xt = load_x(0)
            bias = pers.tile([128, 6], F32, tag="bias")
            nc.sync.dma_start(bias[:], bias_d.ap())
            ident = pers.tile([128, 128], F32R, tag="ident")
            nc.sync.dma_start(ident[:], ident_d.ap())
            vaug = pers.tile([128, T // KT, NH * VW], BF16, tag="vaug")
            nc.sync.dma_start(
                vaug[:].rearrange("p t (h w) -> p t h w", h=NH)[:, :, :, HD:HD + 1],
                vones_d.ap().rearrange("p (t h w) -> p t h w", t=16, h=NH),
            )
            wp = pers.tile([128, 2, NX], BF16, tag="wp")
            nc.sync.dma_start(wp[:], wp_d.ap().rearrange("(c p) n -> p c n", p=128))

            QT = [pers.tile([128, T], BF16, tag=f"qt{i}", name=f"qt{i}")
                  for i in range(2)]
            KTs = [pers.tile([128, T], BF16, tag=f"kt{i}", name=f"kt{i}")
                   for i in range(2)]
            anorm = [pers.tile([128, T], BF16, tag=f"an{i}", name=f"an{i}")
                     for i in range(2)]

            def qkv_proj(qq, xt):
                """Project chunk qq -> QT/KT chunks (bf16) + V^T stages."""
                pq = [bank(0, [128, QC]), bank(1, [128, QC])]
                pk = [bank(2, [128, QC]), bank(3, [128, QC])]
                pv = [bank(4, [128, QC]), bank(5, [128, QC])]
                for j in range(8):
                    rhs = xt[:, j, :]
                    for c2 in range(2):
                        for grp, off in ((pq, 0), (pk, CW), (pv, 2 * CW)):
                            lhsT = wqkv[:, j, off + c2 * 128:off + (c2 + 1) * 128]
                            nc.tensor.matmul(grp[c2][:], lhsT, rhs,
                                             start=(j == 0), stop=(j == 7))
                cs = slice(qq * QC, (qq + 1) * QC)
                vstages = []
                for c2 in range(2):
                    nc.scalar.activation(QT[c2][:, cs], pq[c2][:], Ident,
                                         bias=bias[:, c2:c2 + 1])
                    nc.scalar.activation(KTs[c2][:, cs], pk[c2][:], Ident,
                                         bias=bias[:, 2 + c2:3 + c2])
                    vs = stg.tile([128, QC], F32R, tag="vstage")
                    nc.scalar.activation(vs[:], pv[c2][:], Ident,
                                         bias=bias[:, 4 + c2:5 + c2])
                    vstages.append(vs)
                return vstages

            def v_transpose(qq, vstages):
                """PE-transpose V^T chunk -> V natural (bf16) in vaug."""
                for c2 in range(2):
                    vs = vstages[c2]
                    for blk in range(4):
                        pt_ps = bank(4 + (blk % 2), [128, 128], F32R)
                        nc.tensor.transpose(pt_ps[:],
                                            vs[:, blk * 128:(blk + 1) * 128],
                                            ident[:])
                        tt = qq * 4 + blk
                        dst = vaug[:, tt, c2 * 2 * VW:c2 * 2 * VW + 2 * VW]
                        dst = dst.rearrange("p (h w) -> p h w", h=2)[:, :, 0:HD]
                        src = pt_ps[:].rearrange("p (h w) -> p h w", h=2)
                        nc.vector.tensor_copy(dst, src)

            def attention_pair(hp, qq):
                """Heads (2hp, 2hp+1) for q-chunk qq, interleaved."""
                c2 = hp
                nk = 4 * qq + 4
                qs = slice(qq * QC, (qq + 1) * QC)
                pa = [bank(6, [VW, QC]), bank(7, [VW, QC])]
                pts = {}
                LA = 3

                def s_block(kk, hh):
                    ps_s = bank((2 * kk + hh) % 6, [128, QC])
                    rows = slice(64 * hh, 64 * hh + 64)
                    lhsT = KTs[c2][rows, kk * KT:(kk + 1) * KT]
                    rhs = QT[c2][rows, qs]
                    nc.tensor.matmul(ps_s[:], lhsT, rhs, start=True, stop=True)
                    pt = ptp.tile([128, QC], BF16, tag="pt")
                    nc.scalar.activation(pt[:], ps_s[:], Exp, scale=0.125)
                    if kk >= 4 * qq:  # diagonal band: zero entries with k > q
                        nc.gpsimd.affine_select(
                            pt[:], pt[:], pattern=[[1, QC]],
                            compare_op=mybir.AluOpType.is_ge, fill=0.0,
                            base=qq * QC - kk * KT, channel_multiplier=-1)
                    pts[(kk, hh)] = pt

                def pv_block(kk, hh):
                    h = 2 * hp + hh
                    lhsT = vaug[:, kk, h * VW:(h + 1) * VW]
                    nc.tensor.matmul(pa[hh][:], lhsT, pts.pop((kk, hh))[:],
                                     start=(kk == 0), stop=(kk == nk - 1))

                for kk in range(min(LA, nk)):
                    s_block(kk, 0)
                    s_block(kk, 1)
                for kk in range(nk):
                    if kk + LA < nk:
                        s_block(kk + LA, 0)
                        s_block(kk + LA, 1)
                    pv_block(kk, 0)
                    pv_block(kk, 1)

                for hh in range(2):
                    rows = slice(64 * hh, 64 * hh + 64)
                    recip = rp.tile([1, QC], F32, tag="recip")
                    nc.vector.reciprocal(recip[:], pa[hh][HD:HD + 1, :])
                    rbc = rp.tile([64, QC], F32, tag="rbc")
                    nc.gpsimd.partition_broadcast(rbc[:], recip[:])
                    nc.vector.tensor_mul(anorm[c2][rows, qs],
                                         pa[hh][0:HD, :], rbc[:])

            def c_proj(qq):
                """Output projection for t-rows [qq*QC, (qq+1)*QC)."""
                for i in range(4):
                    tt = qq * 4 + i
                    for nxc in range(2):
                        po = bank((i * 2 + nxc) % 6, [128, QC])
                        for c2 in range(2):
                            nc.tensor.matmul(
                                po[:],
                                anorm[c2][:, tt * 128:(tt + 1) * 128],
                                wp[:, c2, nxc * QC:(nxc + 1) * QC],
                                start=(c2 == 0), stop=(c2 == 1))
                        ot = op.tile([128, QC], F32, tag="ot")
                        if (i * 2 + nxc) % 2 == 0:
                            nc.scalar.copy(ot[:], po[:])
                        else:
                            nc.vector.tensor_copy(ot[:], po[:])
                        nc.sync.dma_start(
                            out_d.ap()[tt * 128:(tt + 1) * 128,
                                       nxc * QC:(nxc + 1) * QC], ot[:])

            # ---- main pipeline over q-chunks ----
            for qq in range(NQC):
                vstages = qkv_proj(qq, xt)
                if qq + 1 < NQC:
                    xt = load_x(qq + 1)
                v_transpose(qq, vstages)
                for hp in range(2):
                    attention_pair(hp, qq)
                c_proj(qq)

    nc.compile()
    return nc


_CACHE = {}


def _get_nc():
    if "nc" not in _CACHE:
        _CACHE["nc"] = _build()
    return _CACHE["nc"]


def kernel(x, w_attn, b_attn, w_proj, b_proj):
    x = np.asarray(x, dtype=np.float32)
    w_attn = np.asarray(w_attn, dtype=np.float32)
    b_attn = np.asarray(b_attn, dtype=np.float32)
    w_proj = np.asarray(w_proj, dtype=np.float32)
    b_proj = np.asarray(b_proj, dtype=np.float32)

    ident = np.eye(128, dtype=np.float32)
    vones = np.ones((128, 64), dtype=NP_BF16)
    in_maps = []
    for core in range(NCORES):
        b, hg = divmod(core, NHG)
        cols = slice(hg * CW, (hg + 1) * CW)
        bias = np.empty((128, 6), dtype=np.float32)
        for qkv_i in range(3):
            bseg = b_attn[qkv_i * NX:][cols]
            bias[:, 2 * qkv_i] = bseg[:128]
            bias[:, 2 * qkv_i + 1] = bseg[128:]
        in_maps.append({
            "xT": np.ascontiguousarray(x[b].T).astype(NP_BF16),
            "wq": np.ascontiguousarray(w_attn[:, cols]).astype(NP_BF16),
            "wk": np.ascontiguousarray(w_attn[:, NX:][:, cols]).astype(NP_BF16),
            "wv": np.ascontiguousarray(w_attn[:, 2 * NX:][:, cols]).astype(NP_BF16),
            "bias": bias,
            "wp": np.ascontiguousarray(w_proj[cols, :]).astype(NP_BF16),
            "ident": ident,
            "vones": vones,
        })

    nc = _get_nc()
    res = run_bass_kernel_spmd(nc, in_maps, core_ids=list(range(NCORES)))
    _CACHE["last_res"] = res
    out = np.empty((B, T, NX), dtype=np.float32)
    for b in range(B):
        acc = res.results[b * NHG]["out_p"].astype(np.float32)
        for hg in range(1, NHG):
            acc = acc + res.results[b * NHG + hg]["out_p"]
        out[b] = acc + b_proj
    return out


# revision 10
# speedup vs baseline: 1.3271x; 1.2279x over previous
"""Trainium2 Bass kernel for causal multi-head attention (GPT-style block).

Reference computation (per batch b):
    qkv = x @ w_attn + b_attn ; q,k,v = split(qkv)
    per head: S = q k^T / sqrt(64); causal mask; P = softmax(S); a = P v
    out = concat_heads(a) @ w_proj + b_proj

Shapes: x (2, 2048, 1024), 16 heads, head_dim 64.

Sharding: 8 cores = 2 batches x 4 head-groups (4 heads each).  Tensor
parallel over heads: each core computes the QKV projection for its 4 heads
(column slice of w_attn), full causal attention for those heads, and its
partial output projection (row slice of w_proj).  Host sums the 4
head-group partials per batch and adds b_proj.

On-chip layouts (per core, T=2048, CW=256=4*64):
    xT      [1024, T]   x transposed (host-prepped bf16), streamed per chunk
    Q^T,K^T [CW, T]     projections with head-channel on partitions (bf16)
    V_aug   [T, 4*65]   V natural layout + ones column per head (the ones
                        column makes the PV matmul also produce the softmax
                        denominator as output row 64)
    S^T     [k, q]      scores transposed: PSUM [128, 512] per (k-tile,
                        q-chunk); P^T = exp(S^T/8) directly feeds PV as the
                        moving operand - no transposes in the hot loop.
Causal masking: off-band blocks are skipped entirely; band blocks are
zeroed post-exp with gpsimd.affine_select (exact: exp(s)*0) on the
otherwise idle GPSIMD engine.  All matmuls are bf16 (1 cycle/row on the
PE; fp32r measured ~1.85 cyc/row on this silicon).  Attention processes
head PAIRS interleaved: the two heads' S^T matmuls land on disjoint
PE row-groups (contract dim 64, base partitions 0/64) and run
concurrently, and the extra independent work keeps the PE dense across
the exp->mask->PV dependency chain (HAM stays warm).
"""

import sys

sys.path.insert(0, "/opt/trn_rl_repo")

import numpy as np
import ml_dtypes

import concourse.bacc as bacc
import concourse.mybir as mybir
import concourse.tile as tile
from concourse.bass_utils import run_bass_kernel_spmd

F32 = mybir.dt.float32
F32R = mybir.dt.float32r
BF16 = mybir.dt.bfloat16
NP_BF16 = np.dtype(ml_dtypes.bfloat16)

B = 2
T = 2048
NX = 1024
H = 16
HD = 64
NCORES = 8
NHG = 4          # head groups (cores per batch)
NH = 4           # heads per core
CW = NH * HD     # 256 channel width per core
QC = 512         # q-chunk (moving dim)
NQC = T // QC    # 4
KT = 128         # k-tile
VW = HD + 1      # 65: V columns + ones column


def _build():
    nc = bacc.Bacc("TRN2", target_bir_lowering=False, debug=False,
                   num_devices=NCORES)
    xT_d = nc.dram_tensor("xT", [NX, T], BF16, kind="ExternalInput")
    wq_d = nc.dram_tensor("wq", [NX, CW], BF16, kind="ExternalInput")
    wk_d = nc.dram_tensor("wk", [NX, CW], BF16, kind="ExternalInput")
    wv_d = nc.dram_tensor("wv", [NX, CW], BF16, kind="ExternalInput")
    bias_d = nc.dram_tensor("bias", [128, 6], F32, kind="ExternalInput")
    wp_d = nc.dram_tensor("wp", [CW, NX], BF16, kind="ExternalInput")
    ident_d = nc.dram_tensor("ident", [128, 128], F32R, kind="ExternalInput")
    vones_d = nc.dram_tensor("vones", [128, 16 * NH], BF16, kind="ExternalInput")
    out_d = nc.dram_tensor("out_p", [T, NX], F32, kind="ExternalOutput")

    Ident = mybir.ActivationFunctionType.Identity
    Exp = mybir.ActivationFunctionType.Exp

    with tile.TileContext(nc) as tc:
        with (
            tc.tile_pool(name="pers", bufs=1) as pers,
            tc.tile_pool(name="xin", bufs=2) as xin,
            tc.tile_pool(name="ps", bufs=1, space="PSUM") as psum,
            tc.tile_pool(name="ptp", bufs=8) as ptp,
            tc.tile_pool(name="stg", bufs=4) as stg,
            tc.tile_pool(name="op", bufs=4) as op,
            tc.tile_pool(name="rp", bufs=4) as rp,
        ):
            def bank(i, shape, dtype=F32):
                return psum.tile(shape, dtype, tag=f"bank{i}", bufs=1,
                                 name=f"bank{i}")

            # ---- persistent tiles; load order: first-needed first ----
            wqkv = pers.tile([128, 8, 3 * CW], BF16, tag="wqkv")
            for w_d, off in ((wq_d, 0), (wk_d, CW), (wv_d, 2 * CW)):
                nc.sync.dma_start(
                    wqkv[:, :, off:off + CW],
                    w_d.ap().rearrange("(j p) c -> p j c", p=128),
                )
            xT_r = xT_d.ap().rearrange("(j p) t -> j p t", p=128)

            def load_x(qq):
                xt = xin.tile([128, 8, QC], BF16, tag="xt")
                for j in range(8):
                    nc.sync.dma_start(xt[:, j, :],
                                      xT_r[j][:, qq * QC:(qq + 1) * QC])
                return xt

            xt = load_x(0)
            bias = pers.tile([128, 6], F32, tag="bias")
            nc.sync.dma_start(bias[:], bias_d.ap())
            ident = pers.tile([128, 128], F32R, tag="ident")
            nc.sync.dma_start(ident[:], ident_d.ap())
            vaug = pers.tile([128, T // KT, NH * VW], BF16, tag="vaug")
            nc.sync.dma_start(
                vaug[:].rearrange("p t (h w) -> p t h w", h=NH)[:, :, :, HD:HD + 1],
                vones_d.ap().rearrange("p (t h w) -> p t h w", t=16, h=NH),
            )
            wp = pers.tile([128, 2, NX], BF16, tag="wp")
            nc.sync.dma_start(wp[:], wp_d.ap().rearrange("(c p) n -> p c n", p=128))

            QT = [pers.tile([128, T], BF16, tag=f"qt{i}", name=f"qt{i}")
                  for i in range(2)]
            KTs = [pers.tile([128, T], BF16, tag=f"kt{i}", name=f"kt{i}")
                   for i in range(2)]
            anorm = [pers.tile([128, T], BF16, tag=f"an{i}", name=f"an{i}")
                     for i in range(2)]

            def qkv_proj(qq, xt):
                """Project chunk qq -> QT/KT chunks (bf16) + V^T stages."""
                pq = [bank(0, [128, QC]), bank(1, [128, QC])]
                pk = [bank(2, [128, QC]), bank(3, [128, QC])]
                pv = [bank(4, [128, QC]), bank(5, [128, QC])]
                for j in range(8):
                    rhs = xt[:, j, :]
                    for c2 in range(2):
                        for grp, off in ((pq, 0), (pk, CW), (pv, 2 * CW)):
                            lhsT = wqkv[:, j, off + c2 * 128:off + (c2 + 1) * 128]
                            nc.tensor.matmul(grp[c2][:], lhsT, rhs,
                                             start=(j == 0), stop=(j == 7))
                cs = slice(qq * QC, (qq + 1) * QC)
                vstages = []
                for c2 in range(2):
                    nc.scalar.activation(QT[c2][:, cs], pq[c2][:], Ident,
                                         bias=bias[:, c2:c2 + 1])
                    nc.scalar.activation(KTs[c2][:, cs], pk[c2][:], Ident,
                                         bias=bias[:, 2 + c2:3 + c2])
                    vs = stg.tile([128, QC], F32R, tag="vstage")
                    nc.scalar.activation(vs[:], pv[c2][:], Ident,
                                         bias=bias[:, 4 + c2:5 + c2])
                    vstages.append(vs)
                return vstages

            def v_transpose(qq, vstages):
                """PE-transpose V^T chunk -> V natural (bf16) in vaug."""
                for c2 in range(2):
                    vs = vstages[c2]
                    for blk in range(4):
                        pt_ps = bank(4 + (blk % 2), [128, 128], F32R)
                        nc.tensor.transpose(pt_ps[:],
                                            vs[:, blk * 128:(blk + 1) * 128],
                                            ident[:])
                        tt = qq * 4 + blk
                        dst = vaug[:, tt, c2 * 2 * VW:c2 * 2 * VW + 2 * VW]
                        dst = dst.rearrange("p (h w) -> p h w", h=2)[:, :, 0:HD]
                        src = pt_ps[:].rearrange("p (h w) -> p h w", h=2)
                        nc.vector.tensor_copy(dst, src)

            def attention_pair(hp, qq):
                """Heads (2hp, 2hp+1) for q-chunk qq, interleaved."""
                c2 = hp
                nk = 4 * qq + 4
                qs = slice(qq * QC, (qq + 1) * QC)
                pa = [bank(6, [VW, QC]), bank(7, [VW, QC])]
                pts = {}
                LA = 3

                def s_block(kk, hh):
                    ps_s = bank((2 * kk + hh) % 6, [128, QC])
                    rows = slice(64 * hh, 64 * hh + 64)
                    lhsT = KTs[c2][rows, kk * KT:(kk + 1) * KT]
                    rhs = QT[c2][rows, qs]
                    nc.tensor.matmul(ps_s[:], lhsT, rhs, start=True, stop=True)
                    pt = ptp.tile([128, QC], BF16, tag="pt")
                    nc.scalar.activation(pt[:], ps_s[:], Exp, scale=0.125)
                    if kk >= 4 * qq:  # diagonal band: zero entries with k > q
                        nc.gpsimd.affine_select(
                            pt[:], pt[:], pattern=[[1, QC]],
                            compare_op=mybir.AluOpType.is_ge, fill=0.0,
                            base=qq * QC - kk * KT, channel_multiplier=-1)
                    pts[(kk, hh)] = pt

                def pv_block(kk, hh):
                    h = 2 * hp + hh
                    lhsT = vaug[:, kk, h * VW:(h + 1) * VW]
                    nc.tensor.matmul(pa[hh][:], lhsT, pts.pop((kk, hh))[:],
                                     start=(kk == 0), stop=(kk == nk - 1))

                for kk in range(min(LA, nk)):
                    s_block(kk, 0)
                    s_block(kk, 1)
                for kk in range(nk):
                    if kk + LA < nk:
                        s_block(kk + LA, 0)
                        s_block(kk + LA, 1)
                    pv_block(kk, 0)
                    pv_block(kk, 1)

                for hh in range(2):
                    rows = slice(64 * hh, 64 * hh + 64)
                    recip = rp.tile([1, QC], F32, tag="recip")
                    nc.vector.reciprocal(recip[:], pa[hh][HD:HD + 1, :])
                    rbc = rp.tile([64, QC], F32, tag="rbc")
                    nc.gpsimd.partition_broadcast(rbc[:], recip[:])
                    nc.vector.tensor_mul(anorm[c2][rows, qs],
                                         pa[hh][0:HD, :], rbc[:])

            def c_proj(qq):
                """Output projection for t-rows [qq*QC, (qq+1)*QC)."""
                for i in range(4):
                    tt = qq * 4 + i
                    for nxc in range(2):
                        po = bank(6 + (i * 2 + nxc) % 2, [128, QC])
                        for c2 in range(2):
                            nc.tensor.matmul(
                                po[:],
                                anorm[c2][:, tt * 128:(tt + 1) * 128],
                                wp[:, c2, nxc * QC:(nxc + 1) * QC],
                                start=(c2 == 0), stop=(c2 == 1))
                        ot = op.tile([128, QC], F32, tag="ot")
                        if (i * 2 + nxc) % 2 == 0:
                            nc.scalar.copy(ot[:], po[:])
                        else:
                            nc.vector.tensor_copy(ot[:], po[:])
                        nc.sync.dma_start(
                            out_d.ap()[tt * 128:(tt + 1) * 128,
                                       nxc * QC:(nxc + 1) * QC], ot[:])

            # ---- main pipeline over q-chunks ----
            # c_proj(qq) is emitted AFTER qkv_proj(qq+1): the PE chews
            # through the next projection while the DVE normalize chain
            # (reciprocal/broadcast/mul) for chunk qq completes, so the
            # in-order PE queue never stalls on anorm.
            vstages = qkv_proj(0, xt)
            for qq in range(NQC):
                v_transpose(qq, vstages)
                for hp in range(2):
                    attention_pair(hp, qq)
                if qq + 1 < NQC:
                    xt = load_x(qq + 1)
                    vstages = qkv_proj(qq + 1, xt)
                c_proj(qq)

    nc.compile()
    return nc


_CACHE = {}


def _get_nc():
    if "nc" not in _CACHE:
        _CACHE["nc"] = _build()
    return _CACHE["nc"]


def kernel(x, w_attn, b_attn, w_proj, b_proj):
    x = np.asarray(x, dtype=np.float32)
    w_attn = np.asarray(w_attn, dtype=np.float32)
    b_attn = np.asarray(b_attn, dtype=np.float32)
    w_proj = np.asarray(w_proj, dtype=np.float32)
    b_proj = np.asarray(b_proj, dtype=np.float32)

    ident = np.eye(128, dtype=np.float32)
    vones = np.ones((128, 64), dtype=NP_BF16)
    in_maps = []
    for core in range(NCORES):
        b, hg = divmod(core, NHG)
        cols = slice(hg * CW, (hg + 1) * CW)
        bias = np.empty((128, 6), dtype=np.float32)
        for qkv_i in range(3):
            bseg = b_attn[qkv_i * NX:][cols]
            bias[:, 2 * qkv_i] = bseg[:128]
            bias[:, 2 * qkv_i + 1] = bseg[128:]
        in_maps.append({
            "xT": np.ascontiguousarray(x[b].T).astype(NP_BF16),
            "wq": np.ascontiguousarray(w_attn[:, cols]).astype(NP_BF16),
            "wk": np.ascontiguousarray(w_attn[:, NX:][:, cols]).astype(NP_BF16),
            "wv": np.ascontiguousarray(w_attn[:, 2 * NX:][:, cols]).astype(NP_BF16),
            "bias": bias,
            "wp": np.ascontiguousarray(w_proj[cols, :]).astype(NP_BF16),
            "ident": ident,
            "vones": vones,
        })

    nc = _get_nc()
    res = run_bass_kernel_spmd(nc, in_maps, core_ids=list(range(NCORES)))
    _CACHE["last_res"] = res
    out = np.empty((B, T, NX), dtype=np.float32)
    for b in range(B):
        acc = res.results[b * NHG]["out_p"].astype(np.float32)
        for hg in range(1, NHG):
            acc = acc + res.results[b * NHG + hg]["out_p"]
        out[b] = acc + b_proj
    return out
